# revision 24
# baseline (speedup 1.0000x reference)
"""BiLSTM+CRF NLL loss kernel for 8 Trainium2 NeuronCores.

Sharding: data-parallel on batch (32 sequences per core). Each core runs the
full BiLSTM + emission + CRF forward/backward partition recurrences for its
shard; host combines per-core partials into the scalar loss.
"""

import numpy as np
import ml_dtypes

import concourse.bass as bass
import concourse.tile as tile
from concourse import mybir
from concourse.bass_utils import run_bass_kernel_spmd

F32 = mybir.dt.float32
BF16 = mybir.dt.bfloat16
I16 = mybir.dt.int16

B, S, V, I, NB = 256, 512, 30000, 100, 19
BOS, EOS = 17, 18
NCORES = 8
BC = B // NCORES          # 32 sequences per core
NT = BC * S               # 16384 tokens per core
KP = I + 1                # 101: embedding dims + ones row (bias aug)
EPAD = 128                # padded embedding row length
RENORM = 16               # CRF renorm interval
TBLK = 8                  # steps per PSUM gate block
NBLK = S // TBLK          # 64
GCH = 8                   # gate chunks: (gamma in [f,i,o,g]) x (dir in [f,b])

_CACHE = {}


def _build_nc():
    nc = bass.Bass()

    # ---- dram I/O ----
    emb_d = nc.dram_tensor("emb_pad", [V, EPAD], BF16, kind="ExternalInput")
    idx_d = nc.dram_tensor("idxs", [128, NT // 128], mybir.dt.int32, kind="ExternalInput")
    eye_d = nc.dram_tensor("eye", [128, 128], BF16, kind="ExternalInput")
    wih_d = nc.dram_tensor("wih", [128, GCH, 128], BF16, kind="ExternalInput")
    whh_d = nc.dram_tensor("whh", [128, GCH, 128], BF16, kind="ExternalInput")
    wc_d = nc.dram_tensor("wc", [128, 2, NB], BF16, kind="ExternalInput")
    bc_d = nc.dram_tensor("bc", [NB, 1], F32, kind="ExternalInput")
    esm_d = nc.dram_tensor("esm", [NB, NB], F32, kind="ExternalInput")
    est_d = nc.dram_tensor("est", [NB, NB], F32, kind="ExternalInput")
    etb_d = nc.dram_tensor("etb", [NB, 1], F32, kind="ExternalInput")
    veb_d = nc.dram_tensor("veb", [NB, BC], F32, kind="ExternalInput")
    ones19_d = nc.dram_tensor("ones19", [NB, 1], F32, kind="ExternalInput")
    one1x19_d = nc.dram_tensor("one1x19", [1, NB], F32, kind="ExternalInput")

    y_out = nc.dram_tensor("y_out", [NB, NT], BF16, kind="ExternalOutput")
    res_out = nc.dram_tensor("res", [4, BC], F32, kind="ExternalOutput")

    SIG = mybir.ActivationFunctionType.Sigmoid
    TANH = mybir.ActivationFunctionType.Tanh
    EXP = mybir.ActivationFunctionType.Exp
    LOG = mybir.ActivationFunctionType.Ln

    with tile.TileContext(nc) as tc:
        with tc.tile_pool(name="big", bufs=1) as bp:
            xeT_f = bp.tile([128, NT], BF16, tag="xeT_f")
            eye_s = bp.tile([128, 128], BF16, tag="eye_s")
            # h storage in round order: col (t+1)*32 = h after step t; col 0 = h(-1)=0
            h_all = bp.tile([128, 2, NT + BC], BF16, tag="h_all")
            Y = bp.tile([NB, NT], BF16, tag="Y")
            idx_f = bp.tile([128, NT // 128], mybir.dt.int32, tag="idx_f")
            wih = bp.tile([128, GCH, 128], BF16, tag="wih")
            whh = bp.tile([128, GCH, 128], BF16, tag="whh")
            wc = bp.tile([128, 2, NB], BF16, tag="wc")
            bc_s = bp.tile([NB, 1], F32, tag="bc_s")
            esm = bp.tile([NB, NB], F32, tag="esm")
            est = bp.tile([NB, NB], F32, tag="est")
            etb = bp.tile([NB, 1], F32, tag="etb")
            veb = bp.tile([NB, BC], F32, tag="veb")
            ones19 = bp.tile([NB, 1], F32, tag="ones19")
            one1x19 = bp.tile([1, NB], F32, tag="one1x19")
            gates_s0 = bp.tile([128, 6, BC], F32, tag="gates_s0")
            gates_s1 = bp.tile([128, 6, BC], F32, tag="gates_s1")
            cell = bp.tile([128, 6, BC], F32, tag="cell")  # [c | gA | gB]
            th0 = bp.tile([128, 2, BC], F32, tag="th0")
            th1 = bp.tile([128, 2, BC], F32, tag="th1")
            u_t = bp.tile([128, 4, BC], F32, tag="u_t")
            Wf = bp.tile([NB, BC], F32, tag="Wf")
            Vb = bp.tile([NB, BC], F32, tag="Vb")
            P2 = bp.tile([NB, BC], F32, tag="P2")
            acc_f = bp.tile([1, BC], F32, tag="acc_f")
            acc_b = bp.tile([1, BC], F32, tag="acc_b")
            rec_s = bp.tile([1, BC], F32, tag="rec_s")
            res_s = bp.tile([4, BC], F32, tag="res_s")

            # ---- loads ----
            nc.sync.dma_start(out=idx_f[:, :], in_=idx_d[:])
            nc.sync.dma_start(out=eye_s[:, :], in_=eye_d[:])
            nc.sync.dma_start(out=wih[:, :, :], in_=wih_d[:])
            nc.sync.dma_start(out=whh[:, :, :], in_=whh_d[:])
            nc.sync.dma_start(out=wc[:, :, :], in_=wc_d[:])
            nc.sync.dma_start(out=bc_s[:, :], in_=bc_d[:])
            nc.sync.dma_start(out=esm[:, :], in_=esm_d[:])
            nc.sync.dma_start(out=est[:, :], in_=est_d[:])
            nc.sync.dma_start(out=etb[:, :], in_=etb_d[:])
            nc.sync.dma_start(out=veb[:, :], in_=veb_d[:])
            nc.sync.dma_start(out=ones19[:, :], in_=ones19_d[:])
            nc.sync.dma_start(out=one1x19[:, :], in_=one1x19_d[:])

            # embedding gather: natural-layout indirect DMA + PE transpose
            with tc.tile_pool(name="gat", bufs=4) as gp, \
                 tc.tile_pool(name="gps", bufs=4, space="PSUM") as gpp:
                for k in range(NT // 128):
                    nat = gp.tile([128, EPAD], BF16, tag="nat")
                    nc.gpsimd.indirect_dma_start(
                        out=nat[:, :], out_offset=None,
                        in_=emb_d[:, :],
                        in_offset=bass.IndirectOffsetOnAxis(
                            ap=idx_f[:, k:k + 1], axis=0),
                    )
                    tp = gpp.tile([128, 128], BF16, tag="tp")
                    nc.tensor.transpose(tp[:, :], nat[:, :], eye_s[:, :])
                    nc.vector.tensor_copy(
                        xeT_f[:, k * 128:(k + 1) * 128], tp[:, :])

            # acc zero-init
            nc.vector.memset(acc_f[:, :], 0.0)
            nc.vector.memset(acc_b[:, :], 0.0)

            # ================= phase A: both LSTMs =================
            with tc.tile_pool(name="psA", bufs=1, space="PSUM") as pa:
                GA = pa.tile([128, GCH, TBLK, BC], F32, tag="GA")
                GB = pa.tile([128, GCH, TBLK, BC], F32, tag="GB")
                gbuf = (GA, GB)

                def bulk(k):
                    G = gbuf[k % 2]
                    xe3 = xeT_f[0:KP, :].rearrange("p (t b) -> p t b", b=BC)
                    for c in range(GCH):
                        if c % 2 == 0:
                            rhs = xe3[:, k * TBLK:(k + 1) * TBLK, :]
                        else:
                            hi = S - 1 - k * TBLK
                            rhs = (xe3[:, hi:hi - TBLK:-1, :]
                                   if hi - TBLK >= 0 else xe3[:, hi::-1, :])
                        nc.tensor.matmul(
                            G[:, c, :, :], wih[0:KP, c, :], rhs,
                            start=True, stop=False, skip_group_check=True,
                        )

                def step(t):
                    G = gbuf[(t // TBLK) % 2]
                    tau = t % TBLK
                    rd = t * BC
                    gs = gates_s0 if t % 2 == 0 else gates_s1
                    th = th0 if t % 2 == 0 else th1
                    gofs = 2 + 2 * (t % 2)  # ghat half in cell
                    if t > 0:
                        for c in range(GCH):
                            d = c % 2
                            nc.tensor.matmul(
                                G[:, c, tau, :], whh[0:I, c, :],
                                h_all[0:I, d, rd:rd + BC],
                                start=False, stop=True, skip_group_check=True,
                            )
                    nc.scalar.activation(gs[:, :, :], G[:, 0:6, tau, :], SIG)
                    nc.scalar.activation(
                        cell[:, gofs:gofs + 2, :].rearrange("p a b -> p (a b)"),
                        G[:, 6:8, tau, :], TANH)
                    if t % 2 == 0:
                        cpair = cell[:, 0:4, :].rearrange("p a b -> p (a b)")
                    else:
                        cpair = cell[:, :, :].rearrange(
                            "p (x y) b -> p x (y b)", x=3)[:, 0::2, :]
                    if t > 0:
                        nc.vector.tensor_mul(
                            u_t[:, :, :].rearrange("p a b -> p (a b)"),
                            gs[:, 0:4, :].rearrange("p a b -> p (a b)"),
                            cpair)
                        nc.vector.tensor_add(
                            cell[:, 0:2, :].rearrange("p a b -> p (a b)"),
                            u_t[:, 0:2, :].rearrange("p a b -> p (a b)"),
                            u_t[:, 2:4, :].rearrange("p a b -> p (a b)"))
                    else:
                        nc.vector.tensor_mul(
                            cell[:, 0:2, :].rearrange("p a b -> p (a b)"),
                            gs[:, 2:4, :].rearrange("p a b -> p (a b)"),
                            cell[:, gofs:gofs + 2, :].rearrange("p a b -> p (a b)"))
                    nc.scalar.activation(
                        th[:, :, :].rearrange("p a b -> p (a b)"),
                        cell[:, 0:2, :].rearrange("p a b -> p (a b)"), TANH)
                    wr = (t + 1) * BC
                    nc.vector.tensor_mul(
                        h_all[:, :, wr:wr + BC], gs[:, 4:6, :], th[:, :, :]
                    )

                bulk(0)
                bulk(1)
                for k in range(NBLK):
                    for t in range(k * TBLK, (k + 1) * TBLK):
                        step(t)
                    if k + 2 < NBLK:
                        bulk(k + 2)

            # ================= phase B: emissions -> Y = exp(em + bc) ========
            with tc.tile_pool(name="psB", bufs=4, space="PSUM") as pb:
                EBLK = 16  # tokens per emission block
                for blk in range(S // EBLK):
                    t0 = blk * EBLK
                    em_ps = pb.tile([NB, EBLK * BC], F32, tag="em_ps")
                    # hf for token t lives at col (t+1)*BC
                    nc.tensor.matmul(
                        em_ps[:, :], wc[0:I, 0, :],
                        h_all[0:I, 0, (t0 + 1) * BC:(t0 + 1 + EBLK) * BC],
                        start=True, stop=False, skip_group_check=True,
                    )
                    # hb for token t lives at round (511-t) -> col (512-t)*BC
                    hb_ap = h_all[0:I, 1, :].rearrange("p (t b) -> p t b", b=BC)
                    nc.tensor.matmul(
                        em_ps[:, :].rearrange("p (t b) -> p t b", b=BC),
                        wc[0:I, 1, :],
                        hb_ap[:, S - t0:S - t0 - EBLK:-1, :],
                        start=False, stop=True, skip_group_check=True,
                    )
                    nc.scalar.activation(
                        Y[:, t0 * BC:(t0 + EBLK) * BC], em_ps[:, :], EXP, bias=bc_s[:, 0:1]
                    )

            nc.sync.dma_start(out=y_out[:], in_=Y[:, :])

            # ================= phase C: CRF partition ======================
            with tc.tile_pool(name="psC", bufs=2, space="PSUM") as pc, \
                 tc.tile_pool(name="psC2", bufs=1, space="PSUM") as pc2:
                # W0 = Y_0 * exp(T[BOS,:])
                nc.vector.tensor_scalar_mul(Wf[:, :], Y[0:NB, 0:BC], etb[:, 0:1])
                # backward init: V = veb * Y_511
                nc.vector.tensor_mul(Vb[:, :], veb[:, :], Y[0:NB, (S - 1) * BC:S * BC])

                def renorm2(w_sb, acc):
                    # correct version: s = ones19^T @ w
                    s_ps = pc2.tile([1, BC], F32, tag="s_ps")
                    nc.tensor.matmul(s_ps[:, :], ones19[:, :], w_sb[:, :],
                                     skip_group_check=True)
                    nc.vector.reciprocal(rec_s[:, :], s_ps[:, :])
                    r_ps = pc2.tile([NB, BC], F32, tag="r_ps")
                    nc.tensor.matmul(r_ps[:, :], one1x19[:, :], rec_s[:, :],
                                     skip_group_check=True)
                    nc.vector.tensor_mul(w_sb[:, :], w_sb[:, :], r_ps[:, :])
                    lg_ps = pc2.tile([1, BC], F32, tag="lg_ps")
                    nc.scalar.activation(lg_ps[:, :], s_ps[:, :], LOG)
                    nc.vector.tensor_add(acc[:, :], acc[:, :], lg_ps[:, :])

                wb_prev = None
                HALF = S // 2
                for r in range(HALF):
                    # forward chain: t = 1..HALF-1
                    if r >= 1:
                        t = r
                        wf_ps = pc.tile([NB, BC], F32, tag="wf_ps")
                        nc.tensor.matmul(wf_ps[:, :], esm[:, :], Wf[:, :],
                                         skip_group_check=True)
                        nc.vector.tensor_mul(
                            Wf[:, :], wf_ps[:, :], Y[0:NB, t * BC:(t + 1) * BC]
                        )
                        if t % RENORM == 0:
                            renorm2(Wf, acc_f)
                    # backward chain: applications t+1 = 511 - r
                    wb_ps = pc.tile([NB, BC], F32, tag="wb_ps")
                    nc.tensor.matmul(wb_ps[:, :], est[:, :], Vb[:, :],
                                     skip_group_check=True)
                    if r < HALF - 1:
                        ty = S - 2 - r  # next Y column for backward chain
                        nc.vector.tensor_mul(
                            Vb[:, :], wb_ps[:, :], Y[0:NB, ty * BC:(ty + 1) * BC]
                        )
                        if r % RENORM == 0 and r > 0:
                            renorm2(Vb, acc_b)
                    wb_prev = wb_ps

                # meet at t=255: P2 = Wf_255 * beta_255
                nc.vector.tensor_mul(P2[:, :], Wf[:, :], wb_prev[:, :])
                dot_ps = pc2.tile([1, BC], F32, tag="s_ps")
                nc.tensor.matmul(dot_ps[:, :], ones19[:, :], P2[:, :],
                                 skip_group_check=True)
                nc.scalar.activation(res_s[0:1, :], dot_ps[:, :], LOG)

            nc.sync.dma_start(out=res_out[0:1], in_=res_s[0:1, :])
            nc.sync.dma_start(out=res_out[1:2], in_=acc_f[:, :])
            nc.sync.dma_start(out=res_out[2:3], in_=acc_b[:, :])

    return nc


def _split_waits(nc):
    """Walrus codegen allows ~1 sync-wait on compute instrs; move excess
    waits onto injected same-engine Drain instructions (which allow many)."""
    from concourse import mybir as mb
    n = 0
    for f in nc.m.functions:
        for blk in f.blocks:
            insts = blk.instructions
            i = 0
            new_list = []
            for ins in insts:
                si = ins.sync_info
                if si is not None and si.on_wait and len(si.on_wait) > 1:
                    keep = list(si.on_wait)[-1:] \
                        if type(ins).__name__ == 'InstDrain' else []
                    for w in list(si.on_wait)[:-1] if keep else list(si.on_wait):
                        d = mb.InstDrain(
                            name=f"{ins.name}-ws{n}", ins=[], outs=[])
                        d.engine = ins.engine
                        d.sync_info = mb.SyncInfo(on_wait=[w], on_update=[])
                        new_list.append(d)
                        n += 1
                    ins.sync_info = mb.SyncInfo(
                        on_wait=keep, on_update=list(si.on_update))
                new_list.append(ins)
            blk.set_instructions(new_list) if hasattr(blk, 'set_instructions') \
                else None
            if not hasattr(blk, 'set_instructions'):
                del insts[:]
                insts.extend(new_list)
    return n


def _wrap16(arr):
    """int idx array [N] -> [128, N//16] int16 with idx i at [i%16, i//16]."""
    n = arr.shape[0]
    out = np.zeros((128, n // 16), np.int16)
    out[0:16, :] = arr.reshape(n // 16, 16).T
    return out


def _prep_host(inputs):
    emb = np.asarray(inputs["emb"], np.float32)
    T = np.asarray(inputs["transitions"], np.float32)
    W1 = np.asarray(inputs["W1"], np.float32)
    b1 = np.asarray(inputs["b1"], np.float32)
    W2 = np.asarray(inputs["W2"], np.float32)
    b2 = np.asarray(inputs["b2"], np.float32)

    emb_pad = np.zeros((V, EPAD), np.float32)
    emb_pad[:, 0:I] = emb
    emb_pad[:, I] = 1.0  # bias-aug ones row

    # gate reorder: pytorch [i,f,g,o] -> ours [f,i,o,g]
    perm = np.concatenate([np.arange(I, 2 * I), np.arange(0, I),
                           np.arange(3 * I, 4 * I), np.arange(2 * I, 3 * I)])

    def pack_dir(Wih, Whh, bih, bhh):
        Wih, Whh = Wih[perm], Whh[perm]
        bias = (bih + bhh)[perm]
        wih = np.zeros((4, 128, 128), np.float32)  # [gamma, k, m]
        whh = np.zeros((4, 128, 128), np.float32)
        for g in range(4):
            wih[g, 0:I, 0:I] = Wih[g * I:(g + 1) * I].T
            wih[g, I, 0:I] = bias[g * I:(g + 1) * I]
            whh[g, 0:I, 0:I] = Whh[g * I:(g + 1) * I].T
        return wih, whh

    wih_f, whh_f = pack_dir(np.asarray(inputs["Wih_f"], np.float32),
                            np.asarray(inputs["Whh_f"], np.float32),
                            np.asarray(inputs["bih_f"], np.float32),
                            np.asarray(inputs["bhh_f"], np.float32))
    wih_b, whh_b = pack_dir(np.asarray(inputs["Wih_b"], np.float32),
                            np.asarray(inputs["Whh_b"], np.float32),
                            np.asarray(inputs["bih_b"], np.float32),
                            np.asarray(inputs["bhh_b"], np.float32))

    wih = np.zeros((128, GCH, 128), np.float32)
    whh = np.zeros((128, GCH, 128), np.float32)
    for g in range(4):
        wih[:, g * 2 + 0, :] = wih_f[g]
        wih[:, g * 2 + 1, :] = wih_b[g]
        whh[:, g * 2 + 0, :] = whh_f[g]
        whh[:, g * 2 + 1, :] = whh_b[g]

    Wc = W2 @ W1                      # [19, 200]
    bcv = W2 @ b1 + b2                # [19]
    wc = np.zeros((128, 2, NB), np.float32)
    wc[0:I, 0, :] = Wc[:, 0:I].T
    wc[0:I, 1, :] = Wc[:, I:2 * I].T

    c0 = float(np.log(np.sum(np.exp(bcv))))
    esm = np.exp(T - c0)
    est = esm.T.copy()
    etb = np.exp(T[BOS, :]).reshape(NB, 1)
    veb = np.broadcast_to(np.exp(T[:, EOS]).reshape(NB, 1), (NB, BC)).copy()

    bf = ml_dtypes.bfloat16
    common = {
        "emb_pad": emb_pad.astype(bf),
        "wih": wih.astype(bf),
        "whh": whh.astype(bf),
        "wc": wc.astype(bf),
        "bc": bcv.reshape(NB, 1).astype(np.float32),
        "esm": esm.astype(np.float32),
        "est": est.astype(np.float32),
        "etb": etb.astype(np.float32),
        "veb": veb.astype(np.float32),
        "ones19": np.ones((NB, 1), np.float32),
        "one1x19": np.ones((1, NB), np.float32),
    }
    return common, c0, bcv


def kernel(**inputs):
    x = np.asarray(inputs["x"]).reshape(B, S).astype(np.int64)
    target = np.asarray(inputs["target"]).reshape(B, S).astype(np.int64)
    T = np.asarray(inputs["transitions"], np.float32)

    common, c0, bcv = _prep_host(inputs)

    common["eye"] = np.eye(128, dtype=ml_dtypes.bfloat16)
    in_maps = []
    for c in range(NCORES):
        xs = x[c * BC:(c + 1) * BC]  # [BC, S]
        # fwd token order: col t*BC + b  -> x[b, t]
        idx_fwd = xs.T.reshape(-1).astype(np.int32)
        idxs = idx_fwd.reshape(NT // 128, 128).T.copy()
        in_maps.append({**common, "idxs": idxs})

    if "nc" not in _CACHE:
        nc0 = _build_nc()
        _split_waits(nc0)
        mybir.codegen_inst_isa_subclasses(nc0)
        _CACHE["nc"] = nc0
    nc = _CACHE["nc"]
    _CACHE["in_maps"] = in_maps

    results = run_bass_kernel_spmd(nc, in_maps, list(range(NCORES))).results

    # host combine
    t_sc = (T[target[:, :-1], target[:, 1:]].sum(1)
            + T[BOS, target[:, 0]] + T[target[:, -1], EOS])  # [B]

    losses = np.zeros(B, np.float64)
    for c in range(NCORES):
        yv = np.asarray(results[c]["y_out"], ml_dtypes.bfloat16).astype(np.float32)
        res = np.asarray(results[c]["res"], np.float32)
        logY = np.log(yv).reshape(NB, S, BC)  # log Y = em + bc
        tg = target[c * BC:(c + 1) * BC]      # [BC, S]
        bi = np.arange(BC)
        e_sc = np.zeros(BC, np.float64)
        for t in range(S):
            e_sc += logY[tg[:, t], t, bi]
        partition = res[0] + res[1] + res[2] + (S - 1) * c0
        losses[c * BC:(c + 1) * BC] = (
            e_sc + t_sc[c * BC:(c + 1) * BC] - partition
        )
    return np.float32(-losses.mean())


# revision 26
# speedup vs baseline: 1.3516x; 1.3516x over previous
"""BiLSTM+CRF NLL loss kernel for 8 Trainium2 NeuronCores.

Sharding: data-parallel on batch (32 sequences per core). Each core runs the
full BiLSTM + emission + CRF forward/backward partition recurrences for its
shard; host combines per-core partials into the scalar loss.
"""

import numpy as np
import ml_dtypes

import concourse.bass as bass
import concourse.tile as tile
from concourse import mybir
from concourse.bass_utils import run_bass_kernel_spmd

F32 = mybir.dt.float32
BF16 = mybir.dt.bfloat16
I16 = mybir.dt.int16

B, S, V, I, NB = 256, 512, 30000, 100, 19
BOS, EOS = 17, 18
NCORES = 8
BC = B // NCORES          # 32 sequences per core
NT = BC * S               # 16384 tokens per core
KP = I + 1                # 101: embedding dims + ones row (bias aug)
EPAD = 128                # padded embedding row length
RENORM = 16               # CRF renorm interval
TBLK = 8                  # steps per PSUM gate block
NBLK = S // TBLK          # 64
GCH = 8                   # gate chunks: (gamma in [f,i,o,g]) x (dir in [f,b])

_CACHE = {}


def _build_nc():
    nc = bass.Bass()

    # ---- dram I/O ----
    emb_d = nc.dram_tensor("emb_pad", [V, EPAD], BF16, kind="ExternalInput")
    idx_d = nc.dram_tensor("idxs", [128, NT // 128], mybir.dt.int32, kind="ExternalInput")
    eye_d = nc.dram_tensor("eye", [128, 128], BF16, kind="ExternalInput")
    wih_d = nc.dram_tensor("wih", [128, GCH, 128], BF16, kind="ExternalInput")
    whh_d = nc.dram_tensor("whh", [128, GCH, 128], BF16, kind="ExternalInput")
    wc_d = nc.dram_tensor("wc", [128, 2, NB], BF16, kind="ExternalInput")
    bc_d = nc.dram_tensor("bc", [NB, 1], F32, kind="ExternalInput")
    esm_d = nc.dram_tensor("esm", [NB, NB], F32, kind="ExternalInput")
    est_d = nc.dram_tensor("est", [NB, NB], F32, kind="ExternalInput")
    etb_d = nc.dram_tensor("etb", [NB, 1], F32, kind="ExternalInput")
    veb_d = nc.dram_tensor("veb", [NB, BC], F32, kind="ExternalInput")
    ones19_d = nc.dram_tensor("ones19", [NB, 1], F32, kind="ExternalInput")
    one1x19_d = nc.dram_tensor("one1x19", [1, NB], F32, kind="ExternalInput")

    y_out = nc.dram_tensor("y_out", [NB, NT], BF16, kind="ExternalOutput")
    res_out = nc.dram_tensor("res", [4, BC], F32, kind="ExternalOutput")

    SIG = mybir.ActivationFunctionType.Sigmoid
    TANH = mybir.ActivationFunctionType.Tanh
    EXP = mybir.ActivationFunctionType.Exp
    LOG = mybir.ActivationFunctionType.Ln

    with tile.TileContext(nc) as tc:
        with tc.tile_pool(name="big", bufs=1) as bp:
            xeT_f = bp.tile([128, NT], BF16, tag="xeT_f")
            eye_s = bp.tile([128, 128], BF16, tag="eye_s")
            # h storage in round order: col (t+1)*32 = h after step t; col 0 = h(-1)=0
            h_all = bp.tile([128, 2, NT + BC], BF16, tag="h_all")
            Y = bp.tile([NB, NT], BF16, tag="Y")
            idx_f = bp.tile([128, NT // 128], mybir.dt.int32, tag="idx_f")
            wih = bp.tile([128, GCH, 128], BF16, tag="wih")
            whh = bp.tile([128, GCH, 128], BF16, tag="whh")
            wc = bp.tile([128, 2, NB], BF16, tag="wc")
            bc_s = bp.tile([NB, 1], F32, tag="bc_s")
            esm = bp.tile([NB, NB], F32, tag="esm")
            est = bp.tile([NB, NB], F32, tag="est")
            etb = bp.tile([NB, 1], F32, tag="etb")
            veb = bp.tile([NB, BC], F32, tag="veb")
            ones19 = bp.tile([NB, 1], F32, tag="ones19")
            one1x19 = bp.tile([1, NB], F32, tag="one1x19")
            gates_s0 = bp.tile([128, 6, BC], F32, tag="gates_s0")
            gates_s1 = bp.tile([128, 6, BC], F32, tag="gates_s1")
            cell = bp.tile([128, 6, BC], F32, tag="cell")  # [c | gA | gB]
            th0 = bp.tile([128, 2, BC], F32, tag="th0")
            th1 = bp.tile([128, 2, BC], F32, tag="th1")
            u_t = bp.tile([128, 4, BC], F32, tag="u_t")
            Wf = bp.tile([NB, BC], F32, tag="Wf")
            Vb = bp.tile([NB, BC], F32, tag="Vb")
            P2 = bp.tile([NB, BC], F32, tag="P2")
            acc_f = bp.tile([1, BC], F32, tag="acc_f")
            acc_b = bp.tile([1, BC], F32, tag="acc_b")
            rec_s = bp.tile([1, BC], F32, tag="rec_s")
            res_s = bp.tile([4, BC], F32, tag="res_s")

            # ---- loads ----
            nc.sync.dma_start(out=idx_f[:, :], in_=idx_d[:])
            nc.sync.dma_start(out=eye_s[:, :], in_=eye_d[:])
            nc.sync.dma_start(out=wih[:, :, :], in_=wih_d[:])
            nc.sync.dma_start(out=whh[:, :, :], in_=whh_d[:])
            nc.sync.dma_start(out=wc[:, :, :], in_=wc_d[:])
            nc.sync.dma_start(out=bc_s[:, :], in_=bc_d[:])
            nc.sync.dma_start(out=esm[:, :], in_=esm_d[:])
            nc.sync.dma_start(out=est[:, :], in_=est_d[:])
            nc.sync.dma_start(out=etb[:, :], in_=etb_d[:])
            nc.sync.dma_start(out=veb[:, :], in_=veb_d[:])
            nc.sync.dma_start(out=ones19[:, :], in_=ones19_d[:])
            nc.sync.dma_start(out=one1x19[:, :], in_=one1x19_d[:])

            # embedding gather: natural-layout indirect DMA + PE transpose
            with tc.tile_pool(name="gat", bufs=4) as gp, \
                 tc.tile_pool(name="gps", bufs=4, space="PSUM") as gpp:
                for k in range(NT // 128):
                    nat = gp.tile([128, EPAD], BF16, tag="nat")
                    nc.gpsimd.indirect_dma_start(
                        out=nat[:, :], out_offset=None,
                        in_=emb_d[:, :],
                        in_offset=bass.IndirectOffsetOnAxis(
                            ap=idx_f[:, k:k + 1], axis=0),
                    )
                    tp = gpp.tile([128, 128], BF16, tag="tp")
                    nc.tensor.transpose(tp[:, :], nat[:, :], eye_s[:, :])
                    nc.vector.tensor_copy(
                        xeT_f[:, k * 128:(k + 1) * 128], tp[:, :])

            # acc zero-init
            nc.vector.memset(acc_f[:, :], 0.0)
            nc.vector.memset(acc_b[:, :], 0.0)

            # ================= phase A: both LSTMs =================
            with tc.tile_pool(name="psA", bufs=1, space="PSUM") as pa:
                GA = pa.tile([128, GCH, TBLK, BC], F32, tag="GA")
                GB = pa.tile([128, GCH, TBLK, BC], F32, tag="GB")
                gbuf = (GA, GB)

                def bulk(k):
                    G = gbuf[k % 2]
                    xe3 = xeT_f[0:KP, :].rearrange("p (t b) -> p t b", b=BC)
                    for c in range(GCH):
                        if c % 2 == 0:
                            rhs = xe3[:, k * TBLK:(k + 1) * TBLK, :]
                        else:
                            hi = S - 1 - k * TBLK
                            rhs = (xe3[:, hi:hi - TBLK:-1, :]
                                   if hi - TBLK >= 0 else xe3[:, hi::-1, :])
                        nc.tensor.matmul(
                            G[:, c, :, :], wih[0:KP, c, :], rhs,
                            start=True, stop=False, skip_group_check=True,
                        )

                def step(t):
                    G = gbuf[(t // TBLK) % 2]
                    tau = t % TBLK
                    rd = t * BC
                    gs = gates_s0 if t % 2 == 0 else gates_s1
                    th = th0 if t % 2 == 0 else th1
                    gofs = 2 + 2 * (t % 2)  # ghat half in cell
                    if t > 0:
                        for c in range(GCH):
                            d = c % 2
                            nc.tensor.matmul(
                                G[:, c, tau, :], whh[0:I, c, :],
                                h_all[0:I, d, rd:rd + BC],
                                start=False, stop=True, skip_group_check=True,
                            )
                    nc.scalar.activation(gs[:, :, :], G[:, 0:6, tau, :], SIG)
                    nc.scalar.activation(
                        cell[:, gofs:gofs + 2, :].rearrange("p a b -> p (a b)"),
                        G[:, 6:8, tau, :], TANH)
                    if t % 2 == 0:
                        cpair = cell[:, 0:4, :].rearrange("p a b -> p (a b)")
                    else:
                        cpair = cell[:, :, :].rearrange(
                            "p (x y) b -> p x (y b)", x=3)[:, 0::2, :]
                    if t > 0:
                        nc.vector.tensor_mul(
                            u_t[:, :, :].rearrange("p a b -> p (a b)"),
                            gs[:, 0:4, :].rearrange("p a b -> p (a b)"),
                            cpair)
                        nc.vector.tensor_add(
                            cell[:, 0:2, :].rearrange("p a b -> p (a b)"),
                            u_t[:, 0:2, :].rearrange("p a b -> p (a b)"),
                            u_t[:, 2:4, :].rearrange("p a b -> p (a b)"))
                    else:
                        nc.vector.tensor_mul(
                            cell[:, 0:2, :].rearrange("p a b -> p (a b)"),
                            gs[:, 2:4, :].rearrange("p a b -> p (a b)"),
                            cell[:, gofs:gofs + 2, :].rearrange("p a b -> p (a b)"))
                    nc.scalar.activation(
                        th[:, :, :].rearrange("p a b -> p (a b)"),
                        cell[:, 0:2, :].rearrange("p a b -> p (a b)"), TANH)
                    wr = (t + 1) * BC
                    nc.vector.tensor_mul(
                        h_all[:, :, wr:wr + BC], gs[:, 4:6, :], th[:, :, :]
                    )

                bulk(0)
                bulk(1)
                for k in range(NBLK):
                    for t in range(k * TBLK, (k + 1) * TBLK):
                        step(t)
                    if k + 2 < NBLK:
                        bulk(k + 2)

            # ================= phase B: emissions -> Y = exp(em + bc) ========
            with tc.tile_pool(name="psB", bufs=4, space="PSUM") as pb:
                EBLK = 16  # tokens per emission block
                for blk in range(S // EBLK):
                    t0 = blk * EBLK
                    em_ps = pb.tile([NB, EBLK * BC], F32, tag="em_ps")
                    # hf for token t lives at col (t+1)*BC
                    nc.tensor.matmul(
                        em_ps[:, :], wc[0:I, 0, :],
                        h_all[0:I, 0, (t0 + 1) * BC:(t0 + 1 + EBLK) * BC],
                        start=True, stop=False, skip_group_check=True,
                    )
                    # hb for token t lives at round (511-t) -> col (512-t)*BC
                    hb_ap = h_all[0:I, 1, :].rearrange("p (t b) -> p t b", b=BC)
                    nc.tensor.matmul(
                        em_ps[:, :].rearrange("p (t b) -> p t b", b=BC),
                        wc[0:I, 1, :],
                        hb_ap[:, S - t0:S - t0 - EBLK:-1, :],
                        start=False, stop=True, skip_group_check=True,
                    )
                    nc.scalar.activation(
                        Y[:, t0 * BC:(t0 + EBLK) * BC], em_ps[:, :], EXP, bias=bc_s[:, 0:1]
                    )

            nc.sync.dma_start(out=y_out[:], in_=Y[:, :])

            # ================= phase C: CRF partition ======================
            with tc.tile_pool(name="psC", bufs=2, space="PSUM") as pc, \
                 tc.tile_pool(name="psC2", bufs=1, space="PSUM") as pc2:
                # W0 = Y_0 * exp(T[BOS,:])
                nc.vector.tensor_scalar_mul(Wf[:, :], Y[0:NB, 0:BC], etb[:, 0:1])
                # backward init: V = veb * Y_511
                nc.vector.tensor_mul(Vb[:, :], veb[:, :], Y[0:NB, (S - 1) * BC:S * BC])

                def renorm2(w_sb, acc):
                    # correct version: s = ones19^T @ w
                    s_ps = pc2.tile([1, BC], F32, tag="s_ps")
                    nc.tensor.matmul(s_ps[:, :], ones19[:, :], w_sb[:, :],
                                     skip_group_check=True)
                    nc.vector.reciprocal(rec_s[:, :], s_ps[:, :])
                    r_ps = pc2.tile([NB, BC], F32, tag="r_ps")
                    nc.tensor.matmul(r_ps[:, :], one1x19[:, :], rec_s[:, :],
                                     skip_group_check=True)
                    nc.vector.tensor_mul(w_sb[:, :], w_sb[:, :], r_ps[:, :])
                    lg_ps = pc2.tile([1, BC], F32, tag="lg_ps")
                    nc.scalar.activation(lg_ps[:, :], s_ps[:, :], LOG)
                    nc.vector.tensor_add(acc[:, :], acc[:, :], lg_ps[:, :])

                wb_prev = None
                HALF = S // 2
                for r in range(HALF):
                    # forward chain: t = 1..HALF-1
                    if r >= 1:
                        t = r
                        wf_ps = pc.tile([NB, BC], F32, tag="wf_ps")
                        nc.tensor.matmul(wf_ps[:, :], esm[:, :], Wf[:, :],
                                         skip_group_check=True)
                        nc.vector.tensor_mul(
                            Wf[:, :], wf_ps[:, :], Y[0:NB, t * BC:(t + 1) * BC]
                        )
                        if t % RENORM == 0:
                            renorm2(Wf, acc_f)
                    # backward chain: applications t+1 = 511 - r
                    wb_ps = pc.tile([NB, BC], F32, tag="wb_ps")
                    nc.tensor.matmul(wb_ps[:, :], est[:, :], Vb[:, :],
                                     skip_group_check=True)
                    if r < HALF - 1:
                        ty = S - 2 - r  # next Y column for backward chain
                        nc.vector.tensor_mul(
                            Vb[:, :], wb_ps[:, :], Y[0:NB, ty * BC:(ty + 1) * BC]
                        )
                        if r % RENORM == 0 and r > 0:
                            renorm2(Vb, acc_b)
                    wb_prev = wb_ps

                # meet at t=255: P2 = Wf_255 * beta_255
                nc.vector.tensor_mul(P2[:, :], Wf[:, :], wb_prev[:, :])
                dot_ps = pc2.tile([1, BC], F32, tag="s_ps")
                nc.tensor.matmul(dot_ps[:, :], ones19[:, :], P2[:, :],
                                 skip_group_check=True)
                nc.scalar.activation(res_s[0:1, :], dot_ps[:, :], LOG)

            nc.sync.dma_start(out=res_out[0:1], in_=res_s[0:1, :])
            nc.sync.dma_start(out=res_out[1:2], in_=acc_f[:, :])
            nc.sync.dma_start(out=res_out[2:3], in_=acc_b[:, :])

    return nc


def _split_waits(nc):
    """Walrus codegen allows ~1 sync-wait on compute instrs; move excess
    waits onto injected same-engine Drain instructions (which allow many)."""
    from concourse import mybir as mb
    n = 0
    for f in nc.m.functions:
        for blk in f.blocks:
            insts = blk.instructions
            i = 0
            new_list = []
            for ins in insts:
                si = ins.sync_info
                if si is not None and si.on_wait and len(si.on_wait) > 1:
                    keep = list(si.on_wait)[-1:] \
                        if type(ins).__name__ == 'InstDrain' else []
                    for w in list(si.on_wait)[:-1] if keep else list(si.on_wait):
                        d = mb.InstDrain(
                            name=f"{ins.name}-ws{n}", ins=[], outs=[])
                        d.engine = ins.engine
                        d.sync_info = mb.SyncInfo(on_wait=[w], on_update=[])
                        new_list.append(d)
                        n += 1
                    ins.sync_info = mb.SyncInfo(
                        on_wait=keep, on_update=list(si.on_update))
                new_list.append(ins)
            blk.set_instructions(new_list) if hasattr(blk, 'set_instructions') \
                else None
            if not hasattr(blk, 'set_instructions'):
                del insts[:]
                insts.extend(new_list)
    return n


def _wrap16(arr):
    """int idx array [N] -> [128, N//16] int16 with idx i at [i%16, i//16]."""
    n = arr.shape[0]
    out = np.zeros((128, n // 16), np.int16)
    out[0:16, :] = arr.reshape(n // 16, 16).T
    return out


def _prep_host(inputs):
    emb = np.asarray(inputs["emb"], np.float32)
    T = np.asarray(inputs["transitions"], np.float32)
    W1 = np.asarray(inputs["W1"], np.float32)
    b1 = np.asarray(inputs["b1"], np.float32)
    W2 = np.asarray(inputs["W2"], np.float32)
    b2 = np.asarray(inputs["b2"], np.float32)

    emb_pad = np.zeros((V, EPAD), np.float32)
    emb_pad[:, 0:I] = emb
    emb_pad[:, I] = 1.0  # bias-aug ones row

    # gate reorder: pytorch [i,f,g,o] -> ours [f,i,o,g]
    perm = np.concatenate([np.arange(I, 2 * I), np.arange(0, I),
                           np.arange(3 * I, 4 * I), np.arange(2 * I, 3 * I)])

    def pack_dir(Wih, Whh, bih, bhh):
        Wih, Whh = Wih[perm], Whh[perm]
        bias = (bih + bhh)[perm]
        wih = np.zeros((4, 128, 128), np.float32)  # [gamma, k, m]
        whh = np.zeros((4, 128, 128), np.float32)
        for g in range(4):
            wih[g, 0:I, 0:I] = Wih[g * I:(g + 1) * I].T
            wih[g, I, 0:I] = bias[g * I:(g + 1) * I]
            whh[g, 0:I, 0:I] = Whh[g * I:(g + 1) * I].T
        return wih, whh

    wih_f, whh_f = pack_dir(np.asarray(inputs["Wih_f"], np.float32),
                            np.asarray(inputs["Whh_f"], np.float32),
                            np.asarray(inputs["bih_f"], np.float32),
                            np.asarray(inputs["bhh_f"], np.float32))
    wih_b, whh_b = pack_dir(np.asarray(inputs["Wih_b"], np.float32),
                            np.asarray(inputs["Whh_b"], np.float32),
                            np.asarray(inputs["bih_b"], np.float32),
                            np.asarray(inputs["bhh_b"], np.float32))

    wih = np.zeros((128, GCH, 128), np.float32)
    whh = np.zeros((128, GCH, 128), np.float32)
    for g in range(4):
        wih[:, g * 2 + 0, :] = wih_f[g]
        wih[:, g * 2 + 1, :] = wih_b[g]
        whh[:, g * 2 + 0, :] = whh_f[g]
        whh[:, g * 2 + 1, :] = whh_b[g]

    Wc = W2 @ W1                      # [19, 200]
    bcv = W2 @ b1 + b2                # [19]
    wc = np.zeros((128, 2, NB), np.float32)
    wc[0:I, 0, :] = Wc[:, 0:I].T
    wc[0:I, 1, :] = Wc[:, I:2 * I].T

    c0 = float(np.log(np.sum(np.exp(bcv))))
    esm = np.exp(T - c0)
    est = esm.T.copy()
    etb = np.exp(T[BOS, :]).reshape(NB, 1)
    veb = np.broadcast_to(np.exp(T[:, EOS]).reshape(NB, 1), (NB, BC)).copy()

    bf = ml_dtypes.bfloat16
    common = {
        "emb_pad": emb_pad.astype(bf),
        "wih": wih.astype(bf),
        "whh": whh.astype(bf),
        "wc": wc.astype(bf),
        "bc": bcv.reshape(NB, 1).astype(np.float32),
        "esm": esm.astype(np.float32),
        "est": est.astype(np.float32),
        "etb": etb.astype(np.float32),
        "veb": veb.astype(np.float32),
        "ones19": np.ones((NB, 1), np.float32),
        "one1x19": np.ones((1, NB), np.float32),
    }
    return common, c0, bcv


def kernel(**inputs):
    x = np.asarray(inputs["x"]).reshape(B, S).astype(np.int64)
    target = np.asarray(inputs["target"]).reshape(B, S).astype(np.int64)
    T = np.asarray(inputs["transitions"], np.float32)

    common, c0, bcv = _prep_host(inputs)

    common["eye"] = np.eye(128, dtype=ml_dtypes.bfloat16)
    in_maps = []
    for c in range(NCORES):
        xs = x[c * BC:(c + 1) * BC]  # [BC, S]
        # fwd token order: col t*BC + b  -> x[b, t]
        idx_fwd = xs.T.reshape(-1).astype(np.int32)
        idxs = idx_fwd.reshape(NT // 128, 128).T.copy()
        in_maps.append({**common, "idxs": idxs})

    if "nc" not in _CACHE:
        nc0 = _build_nc()
        _split_waits(nc0)
        mybir.codegen_inst_isa_subclasses(nc0)
        _CACHE["nc"] = nc0
    nc = _CACHE["nc"]
    _CACHE["in_maps"] = in_maps

    results = run_bass_kernel_spmd(nc, in_maps, list(range(NCORES))).results

    # host combine
    t_sc = (T[target[:, :-1], target[:, 1:]].sum(1)
            + T[BOS, target[:, 0]] + T[target[:, -1], EOS])  # [B]

    losses = np.zeros(B, np.float64)
    for c in range(NCORES):
        yv = np.asarray(results[c]["y_out"], ml_dtypes.bfloat16).astype(np.float32)
        res = np.asarray(results[c]["res"], np.float32)
        logY = np.log(yv).reshape(NB, S, BC)  # log Y = em + bc
        tg = target[c * BC:(c + 1) * BC]      # [BC, S]
        bi = np.arange(BC)
        e_sc = np.zeros(BC, np.float64)
        for t in range(S):
            e_sc += logY[tg[:, t], t, bi]
        partition = res[0] + res[1] + res[2] + (S - 1) * c0
        losses[c * BC:(c + 1) * BC] = (
            e_sc + t_sc[c * BC:(c + 1) * BC] - partition
        )
    return np.float32(-losses.mean())
